# revision 1
# baseline (speedup 1.0000x reference)
"""AttentionPooling (query position 0 only) — Trainium2 Bass/Tile kernel.

Math (per batch n, heads h=8, dh=32, D=256, T=4096):
    q0 = v[n,0,:] @ W_q + b_q                                  (256,)
    scores[h,t] = (1/16) * sum_{j in head h} q0[j] * k[t,j],   k = v @ W_k + b_k
Fold the k-projection into a per-batch "folded query":
    fq[din,h] = sum_{j in head h} W_k[din,j] * q0[j] / 16
    c[h]      = sum_{j in head h} b_k[j]    * q0[j] / 16
    scores[h,t] = sum_din v[t,din] * fq[din,h] + c[h]
    out[h,:]  = sum_t softmax_t(scores[h,:]) * v[t,:]  -> keep cols [32h:32h+32]
This removes the (N*T*D*2D) k-projection entirely; the kernel is then
memory-bound on streaming v once. Softmax is computed without max
subtraction (scores are O(1); exp is safe in fp32) so the normalizer can be
applied after the value accumulation.

Sharding: data-parallel over N across 8 cores (4 batches per core), no
collectives. Each core streams its v shard, PE-transposes v blocks for the
din-contraction (scores), and consumes v in natural layout for the
t-contraction (value einsum).
"""

import sys

if "/opt/trn_rl_repo" not in sys.path:
    sys.path.insert(0, "/opt/trn_rl_repo")

import numpy as np

N_FULL, T, DIN = 32, 4096, 256
H = 8
NCORES = 8
NB = N_FULL // NCORES  # batches per core
TC = 512               # t-chunk processed per iteration
NJ = TC // 128         # 128-row blocks per chunk
NCH = T // TC          # chunks per batch
SCALE = 1.0 / 16.0     # 1/sqrt(D)

import os

# Default to the exact-fp32 configuration: regular fp32 matmuls everywhere
# (transposes via matmul-with-identity). The f32r/transpose-mode variant
# (K_F32R=1 K_TMODE=1) measured 200us vs 264us but costs ~1.5e-4 relative
# error (f32r is a reduced-precision single-pass matmul) and the fp32r
# small-shape matmul variant hung the device once — not worth the risk.
USE_F32R = os.environ.get("K_F32R", "0") == "1"
USE_TMODE = os.environ.get("K_TMODE", "0") == "1"
DEEP_BUFS = os.environ.get("K_BUFS", "1") == "1"

_CACHE = {}


def _build():
    from contextlib import ExitStack

    import concourse.mybir as mybir
    from concourse import bacc
    from concourse.masks import make_identity
    from concourse.tile import TileContext

    fp32 = mybir.dt.float32
    f32r = mybir.dt.float32r if USE_F32R else mybir.dt.float32
    AF = mybir.ActivationFunctionType

    # Bacc (not raw Bass): its compile() pipeline legalizes sync waits
    # (move_matmul_waits_to_ldweights + generate_event_semaphores) — TRN2
    # instructions support at most one embedded wait.
    nc = bacc.Bacc(None, target_bir_lowering=False)
    v_ext = nc.declare_dram_parameter("v", [NB, T, DIN], f32r, isOutput=False)
    w_ext = nc.declare_dram_parameter("W_qk", [DIN, 2 * DIN], fp32, isOutput=False)
    b_ext = nc.declare_dram_parameter("b_qk", [2 * DIN], fp32, isOutput=False)
    # full per-head pooled vectors; host extracts the per-head 32-col slices
    out_ext = nc.declare_dram_parameter("out", [NB, H, DIN], fp32, isOutput=True)

    with TileContext(nc) as tc:
        with ExitStack() as ctx:
            const = ctx.enter_context(tc.tile_pool(name="const", bufs=1))

            ident = const.tile([128, 128], fp32)
            make_identity(nc, ident)

            def pe_transpose(out, in_, identity):
                if USE_TMODE:
                    nc.tensor.transpose(out, in_=in_, identity=identity)
                else:
                    nc.tensor.matmul(
                        out.bitcast(fp32), lhsT=in_.bitcast(fp32),
                        rhs=identity.bitcast(fp32), start=True, stop=True)

            bf16 = mybir.dt.bfloat16
            ident_bf = const.tile([128, 128], bf16)
            # f32r-typed identity for transposing f32r tiles (verifier wants
            # fp32r matmul operands produced by an fp32r-rounding instruction)
            ident_r = const.tile([128, 128], f32r)
            nc.vector.tensor_copy(out=ident_r, in_=ident)
            nc.vector.tensor_copy(out=ident_bf, in_=ident)

            # W_q / W_k as [din_p, kc, dout] (kc = 128-chunk of din)
            wq_sb = const.tile([128, 2, 256], fp32)
            nc.sync.dma_start(
                out=wq_sb, in_=w_ext[:, 0:256].rearrange("(kc p) d -> p kc d", p=128)
            )
            wk_sb = const.tile([128, 2, 256], fp32)
            nc.sync.dma_start(
                out=wk_sb, in_=w_ext[:, 256:512].rearrange("(kc p) d -> p kc d", p=128)
            )
            bq_sb = const.tile([128, 2], fp32)
            nc.sync.dma_start(
                out=bq_sb, in_=b_ext[0:256].rearrange("(kc p) -> p kc", p=128)
            )
            bk_sb = const.tile([128, 2], fp32)
            nc.sync.dma_start(
                out=bk_sb, in_=b_ext[256:512].rearrange("(kc p) -> p kc", p=128)
            )

            # ---- phase 0: per-batch folded queries (all tiny) ----
            with tc.tile_pool(name="ps_prep", bufs=2, space="PSUM") as ps_prep:
                # WkT[j_p, jc, din] = W_k.T via PE transpose
                wkT_sb = const.tile([128, 2, 256], fp32)
                for jc in range(2):
                    pw = ps_prep.tile([128, 256], fp32, tag="pw")
                    for kc in range(2):
                        pe_transpose(
                            pw[:, kc * 128 : (kc + 1) * 128],
                            wk_sb[:, kc, jc * 128 : (jc + 1) * 128],
                            ident,
                        )
                    nc.vector.tensor_copy(out=wkT_sb[:, jc, :], in_=pw)

                # v0T[din_p, kc, n] = v[n, 0, :]  (one DMA per din-chunk)
                v0_sb = const.tile([128, 2, NB], f32r)
                for kc in range(2):
                    nc.sync.dma_start(
                        out=v0_sb[:, kc, :],
                        in_=v_ext[:, 0, kc * 128 : (kc + 1) * 128].rearrange(
                            "n p -> p n"
                        ),
                    )

                # q0[dq_p, dqc, n] = W_q.T @ v0 + b_q
                q0_sb = const.tile([128, 2, NB], fp32)
                for n in range(NB):
                    for dqc in range(2):
                        pq = ps_prep.tile([128, 1], fp32, tag="pq")
                        for kc in range(2):
                            nc.tensor.matmul(
                                pq,
                                lhsT=wq_sb[:, kc, dqc * 128 : (dqc + 1) * 128],
                                rhs=v0_sb[:, kc, n : n + 1].bitcast(fp32),
                                start=(kc == 0),
                                stop=(kc == 1),
                            )
                        nc.scalar.activation(
                            out=q0_sb[:, dqc, n : n + 1],
                            in_=pq,
                            func=AF.Identity,
                            bias=bq_sb[:, dqc : dqc + 1],
                            scale=1.0,
                        )

                # head mask[j_p, jc, h] = SCALE where j = 128*jc + j_p lies in
                # head h's 32-slice, else 0  (j - 32h in [0, 32))
                mask_sb = const.tile([128, 2, H], fp32)
                nc.gpsimd.memset(mask_sb, SCALE)
                nc.gpsimd.affine_select(
                    out=mask_sb,
                    in_=mask_sb,
                    compare_op=mybir.AluOpType.is_ge,
                    fill=0.0,
                    base=0,
                    pattern=[[128, 2], [-32, H]],
                    channel_multiplier=1,
                )
                nc.gpsimd.affine_select(
                    out=mask_sb,
                    in_=mask_sb,
                    compare_op=mybir.AluOpType.is_ge,
                    fill=0.0,
                    base=31,
                    pattern=[[-128, 2], [32, H]],
                    channel_multiplier=-1,
                )

                # q0m[j_p, jc, n*8+h] = mask * q0 (per-partition scalar)
                q0m_sb = const.tile([128, 2, NB * H], fp32)
                for n in range(NB):
                    for jc in range(2):
                        nc.vector.tensor_scalar_mul(
                            q0m_sb[:, jc, n * H : (n + 1) * H],
                            mask_sb[:, jc, :],
                            q0_sb[:, jc, n : n + 1],
                        )

                # fq[din_p, kc, n*8+h] = W_k @ q0m  (lhsT = WkT)
                fq_sb = const.tile([128, 2, NB * H], f32r)
                for n in range(NB):
                    for kc in range(2):
                        pf = ps_prep.tile([128, H], fp32, tag="pf")
                        for jc in range(2):
                            nc.tensor.matmul(
                                pf,
                                lhsT=wkT_sb[:, jc, kc * 128 : (kc + 1) * 128],
                                rhs=q0m_sb[:, jc, n * H : (n + 1) * H],
                                start=(jc == 0),
                                stop=(jc == 1),
                            )
                        nc.vector.tensor_copy(
                            out=fq_sb[:, kc, n * H : (n + 1) * H], in_=pf
                        )

                # c[h, n] = b_k . q0m[:, h]
                c_sb = const.tile([H, NB], fp32)
                for n in range(NB):
                    pc = ps_prep.tile([H, 1], fp32, tag="pc")
                    for jc in range(2):
                        nc.tensor.matmul(
                            pc,
                            lhsT=q0m_sb[:, jc, n * H : (n + 1) * H],
                            rhs=bk_sb[:, jc : jc + 1],
                            start=(jc == 0),
                            stop=(jc == 1),
                        )
                    nc.vector.tensor_copy(out=c_sb[:, n : n + 1], in_=pc)

            # ---- phase 1: stream v ----
            vch = ctx.enter_context(tc.tile_pool(name="vch", bufs=4 if DEEP_BUFS else 3))
            vt = ctx.enter_context(tc.tile_pool(name="vt", bufs=3 if DEEP_BUFS else 2))
            work = ctx.enter_context(tc.tile_pool(name="work", bufs=4 if DEEP_BUFS else 3))
            ps_vt = ctx.enter_context(tc.tile_pool(name="ps_vt", bufs=2, space="PSUM"))
            ps_s = ctx.enter_context(tc.tile_pool(name="ps_s", bufs=2, space="PSUM"))
            ps_et = ctx.enter_context(tc.tile_pool(name="ps_et", bufs=1, space="PSUM"))
            ps_out = ctx.enter_context(tc.tile_pool(name="ps_out", bufs=1, space="PSUM"))

            res_sb = const.tile([H, NB, DIN], fp32)

            for n in range(NB):
                out_acc = ps_out.tile([H, 256], fp32, tag="oacc")
                se_sb = work.tile([H, NCH], fp32, tag="se")
                for ci in range(NCH):
                    t0 = ci * TC
                    # natural-layout chunk: [t_p, j, din], t = t0 + 128*j + t_p
                    vch_sb = vch.tile([128, NJ, DIN], f32r, tag="vch")
                    nc.sync.dma_start(
                        out=vch_sb,
                        in_=v_ext[n, t0 : t0 + TC, :].rearrange(
                            "(j p) d -> p j d", p=128
                        ),
                    )
                    # transpose: vT[din_p, kc, t-chunk] (transpose-mode: single
                    # pass, no fp32 hi/lo split)
                    pvt = ps_vt.tile([128, 2, TC], f32r, tag="pvt")
                    for j in range(NJ):
                        for kc in range(2):
                            pe_transpose(
                                pvt[:, kc, j * 128 : (j + 1) * 128],
                                vch_sb[:, j, kc * 128 : (kc + 1) * 128],
                                ident_r,
                            )
                    vt_sb = vt.tile([128, 2, TC], f32r, tag="vt")
                    nc.vector.tensor_copy(out=vt_sb, in_=pvt)

                    # scores[h, t-chunk] (float32r: single-pass fp32 matmul)
                    ps = ps_s.tile([H, TC], fp32, tag="ps")
                    for kc in range(2):
                        nc.tensor.matmul(
                            ps,
                            lhsT=fq_sb[:, kc, n * H : (n + 1) * H],
                            rhs=vt_sb[:, kc, :],
                            start=(kc == 0),
                            stop=(kc == 1),
                        )
                    # e = exp(scores + c); se accumulates sum_t per chunk
                    e_sb = work.tile([H, TC], fp32, tag="e")
                    nc.scalar.activation(
                        out=e_sb,
                        in_=ps,
                        func=AF.Exp,
                        bias=c_sb[:, n : n + 1],
                        scale=1.0,
                        accum_out=se_sb[:, ci : ci + 1],
                    )
                    # eT[t_p, j, h] via PE transpose
                    pet = ps_et.tile([128, NJ, H], fp32, tag="pet")
                    for j in range(NJ):
                        nc.tensor.matmul(
                            pet[:, j, :],
                            lhsT=e_sb[:, j * 128 : (j + 1) * 128],
                            rhs=ident[0:8, 0:8],
                            start=True,
                            stop=True,
                        )
                    et_sb = work.tile([128, NJ, H], f32r, tag="et")
                    nc.vector.tensor_copy(out=et_sb, in_=pet)

                    # value accumulation: out_acc[h, :] += e[h, t] * v[t, :]
                    for j in range(NJ):
                        nc.tensor.matmul(
                            out_acc,
                            lhsT=et_sb[:, j, :],
                            rhs=vch_sb[:, j, :],
                            start=(ci == 0 and j == 0),
                            stop=(ci == NCH - 1 and j == NJ - 1),
                        )

                # normalize by the softmax denominator
                setot = work.tile([H, 1], fp32, tag="setot")
                nc.vector.reduce_sum(out=setot, in_=se_sb, axis=mybir.AxisListType.X)
                rec = work.tile([H, 1], fp32, tag="rec")
                nc.vector.reciprocal(out=rec, in_=setot)
                nc.vector.tensor_scalar_mul(res_sb[:, n, :], out_acc, rec)

            nc.sync.dma_start(
                out=out_ext[:, :, :].rearrange("n h d -> h n d"), in_=res_sb
            )

    nc.compile()
    return nc


def _get_nc():
    if "nc" not in _CACHE:
        _CACHE["nc"] = _build()
    return _CACHE["nc"]


def _run(inputs, trace=False):
    from concourse.bass_utils import run_bass_kernel_spmd

    v = np.ascontiguousarray(np.asarray(inputs["v"], dtype=np.float32))
    w = np.ascontiguousarray(np.asarray(inputs["W_qk"], dtype=np.float32))
    b = np.ascontiguousarray(np.asarray(inputs["b_qk"], dtype=np.float32))
    nc = _get_nc()
    in_maps = [
        {"v": v[c * NB : (c + 1) * NB], "W_qk": w, "b_qk": b} for c in range(NCORES)
    ]
    res = run_bass_kernel_spmd(nc, in_maps, list(range(NCORES)), trace=trace)
    full = np.concatenate(
        [res.results[c]["out"] for c in range(NCORES)], axis=0
    )  # [N, H, DIN]
    # out[n, 32h + i] = full[n, h, 32h + i]
    fh = full.reshape(N_FULL, H, H, 32)  # [n, h, h', i]
    out = np.ascontiguousarray(
        fh[:, np.arange(H), np.arange(H), :].reshape(N_FULL, DIN)
    ).astype(np.float32)
    return out, res


def kernel(**inputs) -> np.ndarray:
    return _run(inputs, trace=False)[0]



# revision 9
# speedup vs baseline: 2.8809x; 2.8809x over previous
"""AttentionPooling (query position 0 only) — Trainium2 Bass/Tile kernel.

Math (per batch n, heads h=8, dh=32, D=256, T=4096):
    q0 = v[n,0,:] @ W_q + b_q                                  (256,)
    scores[t,h] = (1/16) * sum_{j in head h} q0[j] * k[t,j],   k = v @ W_k + b_k
Fold the k-projection into a per-batch "folded query":
    fq[din,h] = sum_{j in head h} W_k[din,j] * q0[j] / 16
    scores[t,h] = sum_din v[t,din] * fq[din,h]   (+ const(h) which CANCELS in
    softmax since it is uniform over t — so it is dropped entirely)
    out[h,:] = sum_t softmax_t(scores[:,h]) * v[t,:] -> keep cols [32h:32h+32]

Performance structure (per core: 4 batches, v shard 16.8 MB, HBM floor
~47us @358GB/s):
  - All streaming matmuls in bf16 (1-pass PE + fast weight load; fp32 is
    2-pass). l2 rel err ~2e-3, well under the 2e-2 gate.
  - scores are computed TRANSPOSED [t,h] with v^T blocks as the stationary
    operand (weights) and the tiny fq (8 cols) streaming: LDW-dominated
    ~64cyc/block instead of streaming 128 cols. exp output then lands in
    natural t-major layout, so no exp-transpose is needed and the value
    matmul (again v as weights, e streaming 8 cols) consumes it directly.
  - v is DMA'd in [p j] d layout: each partition line is 8KB contiguous HBM
    (vs 1KB before) for near-peak DMA efficiency; 1MB per chunk.
  - softmax denominator via a rank-1 ones matmul accumulated in PSUM.
  - per-chunk downcast fp32->bf16 on the scalar engine, software-pipelined
    one chunk ahead so the PE never waits on it.
  - normalization (divide by Z) and per-head column extraction on host
    (tiny: 32x256 output).

Sharding: data-parallel over N across 8 cores (4 batches/core), no
collectives.
"""

import sys

if "/opt/trn_rl_repo" not in sys.path:
    sys.path.insert(0, "/opt/trn_rl_repo")

import numpy as np

N_FULL, T, DIN = 32, 4096, 256
H = 8
NCORES = 8
NB = N_FULL // NCORES  # batches per core
TC = 1024              # t-chunk processed per iteration
NJ = TC // 128         # rows per partition line (t = t0 + p*NJ + j)
NCH = T // TC          # chunks per batch
SCALE = 1.0 / 16.0     # 1/sqrt(D)

_CACHE = {}


def _build():
    from contextlib import ExitStack

    import concourse.mybir as mybir
    from concourse import bacc
    from concourse.masks import make_identity
    from concourse.tile import TileContext

    fp32 = mybir.dt.float32
    bf16 = mybir.dt.bfloat16
    AF = mybir.ActivationFunctionType

    nc = bacc.Bacc(None, target_bir_lowering=False)
    v_ext = nc.declare_dram_parameter("v", [NB, T, DIN], fp32, isOutput=False)
    w_ext = nc.declare_dram_parameter("W_qk", [DIN, 2 * DIN], fp32, isOutput=False)
    b_ext = nc.declare_dram_parameter("b_qk", [2 * DIN], fp32, isOutput=False)
    # unnormalized pooled values: acc[p, n, db*8+h] = sum_t e[t,h] v[t, db*128+p]
    acc_ext = nc.declare_dram_parameter("acc", [128, NB, 2 * H], fp32, isOutput=True)
    # softmax denominators, per (j, h) partial: z[0, n, j*8+h]
    z_ext = nc.declare_dram_parameter("z", [1, NB, NJ * H], fp32, isOutput=True)

    with TileContext(nc) as tc:
        with ExitStack() as ctx:
            const = ctx.enter_context(tc.tile_pool(name="const", bufs=1))

            ident = const.tile([128, 128], fp32)
            make_identity(nc, ident)
            ident_bf = const.tile([128, 128], bf16)
            nc.vector.tensor_copy(out=ident_bf, in_=ident)
            ones_f = const.tile([128, 1], fp32)
            nc.gpsimd.memset(ones_f, 1.0)
            ones_bf = const.tile([128, 1], bf16)
            nc.vector.tensor_copy(out=ones_bf, in_=ones_f)

            # W_q / W_k as [din_p, kc, dout] (kc = 128-chunk of din)
            wq_sb = const.tile([128, 2, 256], fp32)
            nc.sync.dma_start(
                out=wq_sb, in_=w_ext[:, 0:256].rearrange("(kc p) d -> p kc d", p=128)
            )
            wk_sb = const.tile([128, 2, 256], fp32)
            nc.sync.dma_start(
                out=wk_sb, in_=w_ext[:, 256:512].rearrange("(kc p) d -> p kc d", p=128)
            )
            bq_sb = const.tile([128, 2], fp32)
            nc.sync.dma_start(
                out=bq_sb, in_=b_ext[0:256].rearrange("(kc p) -> p kc", p=128)
            )
            # v0T[din_p, kc, n] = v[n, 0, :]
            v0_sb = const.tile([128, 2, NB], fp32)
            for kc in range(2):
                nc.sync.dma_start(
                    out=v0_sb[:, kc, :],
                    in_=v_ext[:, 0, kc * 128 : (kc + 1) * 128].rearrange("n p -> p n"),
                )

            # ---- phase 0: per-batch folded queries (all tiny, fp32) ----
            with tc.tile_pool(name="ps_prep", bufs=2, space="PSUM") as ps_prep:
                # WkT[j_p, jc, din] = W_k.T via PE transpose (matmul w/ identity)
                wkT_sb = const.tile([128, 2, 256], fp32)
                for jc in range(2):
                    pw = ps_prep.tile([128, 256], fp32, tag="pw")
                    for kc in range(2):
                        nc.tensor.matmul(
                            pw[:, kc * 128 : (kc + 1) * 128],
                            lhsT=wk_sb[:, kc, jc * 128 : (jc + 1) * 128],
                            rhs=ident,
                            start=True,
                            stop=True,
                        )
                    nc.vector.tensor_copy(out=wkT_sb[:, jc, :], in_=pw)

                # q0[dq_p, dqc, n] = W_q.T @ v0 + b_q   (batched over n)
                q0_sb = const.tile([128, 2, NB], fp32)
                for dqc in range(2):
                    pq = ps_prep.tile([128, NB], fp32, tag="pq")
                    for kc in range(2):
                        nc.tensor.matmul(
                            pq,
                            lhsT=wq_sb[:, kc, dqc * 128 : (dqc + 1) * 128],
                            rhs=v0_sb[:, kc, :],
                            start=(kc == 0),
                            stop=(kc == 1),
                        )
                    nc.scalar.activation(
                        out=q0_sb[:, dqc, :],
                        in_=pq,
                        func=AF.Identity,
                        bias=bq_sb[:, dqc : dqc + 1],
                        scale=1.0,
                    )

                # head mask[j_p, jc, h] = SCALE where j = 128*jc + j_p lies in
                # head h's 32-slice, else 0  (j - 32h in [0, 32))
                mask_sb = const.tile([128, 2, H], fp32)
                nc.gpsimd.memset(mask_sb, SCALE)
                nc.gpsimd.affine_select(
                    out=mask_sb,
                    in_=mask_sb,
                    compare_op=mybir.AluOpType.is_ge,
                    fill=0.0,
                    base=0,
                    pattern=[[128, 2], [-32, H]],
                    channel_multiplier=1,
                )
                nc.gpsimd.affine_select(
                    out=mask_sb,
                    in_=mask_sb,
                    compare_op=mybir.AluOpType.is_ge,
                    fill=0.0,
                    base=31,
                    pattern=[[-128, 2], [32, H]],
                    channel_multiplier=-1,
                )

                # q0m[j_p, jc, n*8+h] = mask * q0 (per-partition scalar)
                q0m_sb = const.tile([128, 2, NB * H], fp32)
                for n in range(NB):
                    for jc in range(2):
                        nc.vector.tensor_scalar_mul(
                            q0m_sb[:, jc, n * H : (n + 1) * H],
                            mask_sb[:, jc, :],
                            q0_sb[:, jc, n : n + 1],
                        )

                # fq[din_p, kc, n*8+h] = W_k @ q0m  (lhsT = WkT), cast to bf16
                fq_bf = const.tile([128, 2, NB * H], bf16)
                for kc in range(2):
                    pf = ps_prep.tile([128, NB * H], fp32, tag="pf")
                    for jc in range(2):
                        nc.tensor.matmul(
                            pf,
                            lhsT=wkT_sb[:, jc, kc * 128 : (kc + 1) * 128],
                            rhs=q0m_sb[:, jc, :],
                            start=(jc == 0),
                            stop=(jc == 1),
                        )
                    nc.vector.tensor_copy(out=fq_bf[:, kc, :], in_=pf)

            res_sb = const.tile([128, NB, 2 * H], fp32)
            zres_sb = const.tile([1, NB, NJ * H], fp32)

            # ---- phase 1: stream v ----
            # PSUM discipline: every accumulation group is chunk-local and
            # groups sharing a bank run strictly sequentially — a group START
            # clears the has_written bits of its WHOLE bank, so interleaving
            # two open groups in one bank silently turns accumulates into
            # overwrites. Cross-chunk accumulation happens in SBUF (DVE adds).
            # All PSUM tiles are padded to a full 2KB bank.
            vch = ctx.enter_context(tc.tile_pool(name="vch", bufs=3))
            vbp = ctx.enter_context(tc.tile_pool(name="vb", bufs=3))
            vtp = ctx.enter_context(tc.tile_pool(name="vt", bufs=2))
            epl = ctx.enter_context(tc.tile_pool(name="e", bufs=2))
            ps_vt = ctx.enter_context(tc.tile_pool(name="ps_vt", bufs=2, space="PSUM"))
            ps_st = ctx.enter_context(tc.tile_pool(name="ps_st", bufs=2, space="PSUM"))
            ps_vz = ctx.enter_context(tc.tile_pool(name="ps_vz", bufs=2, space="PSUM"))

            def load(n, ci):
                t0 = ci * TC
                vc = vch.tile([128, NJ, DIN], fp32, tag="vch")
                # partition p <- rows t0+NJ*p .. t0+NJ*p+NJ-1: 8KB contiguous
                nc.sync.dma_start(
                    out=vc,
                    in_=v_ext[n, t0 : t0 + TC, :].rearrange("(p j) d -> p j d", p=128),
                )
                vb = vbp.tile([128, NJ, DIN], bf16, tag="vb")
                nc.scalar.activation(out=vb, in_=vc, func=AF.Copy)
                return vb

            def process(n, ci, vb):
                # v^T for this chunk: vt[d_p, kc, j*128+p] (bf16 PSUM transposes)
                vt_sb = vtp.tile([128, 2, TC], bf16, tag="vt")
                for kc in range(2):
                    pv = ps_vt.tile([128, TC], bf16, tag="pv")
                    for j in range(NJ):
                        nc.tensor.transpose(
                            pv[:, j * 128 : (j + 1) * 128],
                            in_=vb[:, j, kc * 128 : (kc + 1) * 128],
                            identity=ident_bf,
                        )
                    nc.vector.tensor_copy(out=vt_sb[:, kc, :], in_=pv)

                # scoresT[t_p, j*8+h]: v^T blocks stationary, fq streams (N=8)
                sT = ps_st.tile(
                    [128, NJ * H], fp32, tag="st", padded_shape=[128, 512]
                )
                for j in range(NJ):
                    for kc in range(2):
                        nc.tensor.matmul(
                            sT[:, j * H : (j + 1) * H],
                            lhsT=vt_sb[:, kc, j * 128 : (j + 1) * 128],
                            rhs=fq_bf[:, kc, n * H : (n + 1) * H],
                            start=(kc == 0),
                            stop=(kc == 1),
                        )

                # e = exp(scores) in natural t-layout, bf16
                e_bf = epl.tile([128, NJ * H], bf16, tag="e")
                nc.scalar.activation(out=e_bf, in_=sT, func=AF.Exp)

                # chunk-local value + denominator accumulators (one bank):
                # vz[:, db*8+h] = sum_t v[t, db*128+p] e[t, h]
                # vz[0, 16 + j*8+h] = sum_p e[p, j, h]
                vz = ps_vz.tile(
                    [128, 2 * H + NJ * H], fp32, tag="vz", padded_shape=[128, 512]
                )
                for db in range(2):
                    for j in range(NJ):
                        nc.tensor.matmul(
                            vz[:, db * H : (db + 1) * H],
                            lhsT=vb[:, j, db * 128 : (db + 1) * 128],
                            rhs=e_bf[:, j * H : (j + 1) * H],
                            start=(j == 0),
                            stop=(j == NJ - 1),
                        )
                nc.tensor.matmul(
                    vz[0:1, 2 * H : 2 * H + NJ * H],
                    lhsT=ones_bf,
                    rhs=e_bf,
                    start=True,
                    stop=True,
                )

                # cross-chunk accumulation in SBUF
                if ci == 0:
                    nc.vector.tensor_copy(out=res_sb[:, n, :], in_=vz[:, 0 : 2 * H])
                    nc.vector.tensor_copy(
                        out=zres_sb[:, n, :], in_=vz[0:1, 2 * H : 2 * H + NJ * H]
                    )
                else:
                    nc.vector.tensor_add(
                        out=res_sb[:, n, :],
                        in0=res_sb[:, n, :],
                        in1=vz[:, 0 : 2 * H],
                    )
                    nc.vector.tensor_add(
                        out=zres_sb[:, n, :],
                        in0=zres_sb[:, n, :],
                        in1=vz[0:1, 2 * H : 2 * H + NJ * H],
                    )

            # software pipeline: downcast for chunk i+1 is issued before the
            # compute of chunk i so the PE never waits on the scalar engine
            staged = None
            for n in range(NB):
                for ci in range(NCH):
                    vb = load(n, ci)
                    if staged is not None:
                        process(*staged)
                    staged = (n, ci, vb)
            process(*staged)

            nc.sync.dma_start(out=acc_ext[:, :, :], in_=res_sb)
            nc.sync.dma_start(out=z_ext[:, :, :], in_=zres_sb)

    nc.compile()
    return nc


def _get_nc():
    if "nc" not in _CACHE:
        _CACHE["nc"] = _build()
    return _CACHE["nc"]


def _run(inputs, trace=False):
    from concourse.bass_utils import run_bass_kernel_spmd

    v = np.ascontiguousarray(np.asarray(inputs["v"], dtype=np.float32))
    w = np.ascontiguousarray(np.asarray(inputs["W_qk"], dtype=np.float32))
    b = np.ascontiguousarray(np.asarray(inputs["b_qk"], dtype=np.float32))
    nc = _get_nc()
    in_maps = [
        {"v": v[c * NB : (c + 1) * NB], "W_qk": w, "b_qk": b} for c in range(NCORES)
    ]
    res = run_bass_kernel_spmd(nc, in_maps, list(range(NCORES)), trace=trace)

    d = np.arange(DIN)
    p, cb, h = d % 128, d // 128, d // 32
    out = np.empty((N_FULL, DIN), dtype=np.float32)
    for c in range(NCORES):
        acc = res.results[c]["acc"]          # (128, NB, 2*H)
        z = res.results[c]["z"][0]           # (NB, NJ*H)
        Z = z.reshape(NB, NJ, H).sum(axis=1)  # (NB, H)
        sel = acc[p, :, cb * H + h]          # (DIN, NB)
        out[c * NB : (c + 1) * NB] = (sel / Z[:, h].T).T
    return out, res


def kernel(**inputs) -> np.ndarray:
    return _run(inputs, trace=False)[0]
